# revision 11
# baseline (speedup 1.0000x reference)
"""CoAttLayer Trainium2 kernel (fp8 DoubleRow version).

Data-parallel over batch: 64 batches -> 8 NeuronCores x 8 batches.
Per batch (T = N = 1024, d = 64, k = 128):
    A  = R @ Wl                 (t, d)     [computed as A^T = Wl^T @ R^T]
    L  = tanh(A @ P^T)          (t, n)     fp8 DoubleRow (contraction d=64 as 2x32)
    X  = R^T @ L                (d, n)     fp8 DoubleRow (contraction t, 4 tile-pairs)
    LT = L^T                    (n, t)     PE fp8 transposes of L
    Y  = P^T @ L^T              (d, t)     fp8 DoubleRow over LT (contraction n)
    Hp = tanh([Wp Wr] @ [P^T; X])   (k, n)
    Hr = tanh([Wr Wp] @ [R^T; Y])   (k, t)
    logits_p = whp @ Hp, logits_r = whr @ Hr   (row matmuls, F=512)
    Ap = softmax(logits_p), Ar = softmax(logits_r)  (exp, no max-sub: |logit|<~5)
    out = [P^T @ Ap ; R^T @ Ar]     (2d,)

All d-contractions keep operands on partitions 0-63.  The four big GEMMs
(L, X, Y and the L-transposes) run in fp8e4; everything fp32-sensitive
(inputs into A, the H combine, pooling) stays f32r/fp16.  tanh runs once
per L element (ScalarE), straight to fp8.
"""

import numpy as np
from contextlib import ExitStack

B, T, N, D, K = 64, 1024, 1024, 64, 128
NCORES = 8
BL = B // NCORES  # batches per core
NT = T // 128     # 8 t-tiles
NN = N // 128     # 8 n-tiles

_CACHE = {}


def _build():
    import concourse.tile as tile
    from concourse import bacc, mybir
    from concourse.masks import make_identity

    f32 = mybir.dt.float32
    f32r = mybir.dt.float32r
    f16 = mybir.dt.float16
    f8 = mybir.dt.float8e4
    Tanh = mybir.ActivationFunctionType.Tanh
    Exp = mybir.ActivationFunctionType.Exp
    DR = mybir.MatmulPerfMode.DoubleRow

    nc = bacc.Bacc(trn_type="TRN2")

    rv = nc.dram_tensor("review_seq", (BL, T, D), f32r, kind="ExternalInput")
    po = nc.dram_tensor("post_seq", (BL, N, D), f32r, kind="ExternalInput")
    wl = nc.dram_tensor("Wl", (D, D), f32r, kind="ExternalInput")
    wr = nc.dram_tensor("Wr", (K, D), f32r, kind="ExternalInput")
    wp = nc.dram_tensor("Wp", (K, D), f32r, kind="ExternalInput")
    whr = nc.dram_tensor("whr", (1, K), f32, kind="ExternalInput")
    whp = nc.dram_tensor("whp", (1, K), f32, kind="ExternalInput")
    out = nc.dram_tensor("out", (BL, 2 * D), f32, kind="ExternalOutput")

    with tile.TileContext(nc) as tc, ExitStack() as ctx:
        singles = ctx.enter_context(tc.tile_pool(name="singles", bufs=1))
        sb = ctx.enter_context(tc.tile_pool(name="sb", bufs=2))
        # PSUM pools: pl(2x2K) + pt8(2x2K) + pxy(2x2K) + ptr(2x2K) = 16K/partition
        pl = ctx.enter_context(tc.tile_pool(name="pl", bufs=2, space="PSUM"))
        pt8 = ctx.enter_context(tc.tile_pool(name="pt8", bufs=2, space="PSUM"))
        pxy = ctx.enter_context(tc.tile_pool(name="pxy", bufs=2, space="PSUM"))
        ptr = ctx.enter_context(tc.tile_pool(name="ptr", bufs=2, space="PSUM"))

        # ---- per-core constants -------------------------------------------
        ident32 = singles.tile([128, 128], f32)
        make_identity(nc, ident32)
        ident = singles.tile([128, 128], f32r)
        nc.vector.tensor_copy(ident, ident32)
        ident8 = singles.tile([128, 128], f8)
        nc.vector.tensor_copy(ident8, ident32)
        ident16 = singles.tile([128, 128], f16)
        nc.vector.tensor_copy(ident16, ident32)
        one11 = singles.tile([1, 1], f32)
        nc.vector.memset(one11, 1.0)
        one11h = singles.tile([1, 1], f16)
        nc.vector.memset(one11h, 1.0)

        wl_sb = singles.tile([64, 64], f32r)
        nc.sync.dma_start(out=wl_sb, in_=wl[:, :])
        wr_sb = singles.tile([128, 64], f32r)
        nc.sync.dma_start(out=wr_sb, in_=wr[:, :])
        wp_sb = singles.tile([128, 64], f32r)
        nc.sync.dma_start(out=wp_sb, in_=wp[:, :])
        whp_sb = singles.tile([1, 128], f32)
        nc.sync.dma_start(out=whp_sb, in_=whp[:, :])
        whr_sb = singles.tile([1, 128], f32)
        nc.sync.dma_start(out=whr_sb, in_=whr[:, :])

        # [Wp^T; Wr^T] and [Wr^T; Wp^T] stacks (128, 128) f32r on partitions,
        # whT (128, 2) f16 columns.
        ps_w = ptr.tile([128, 512], f32r, tag="ptr")
        nc.tensor.transpose(ps_w[0:64, 0:128], wp_sb, ident)
        nc.tensor.transpose(ps_w[0:64, 128:256], wr_sb, ident)
        WprT = singles.tile([128, 128], f32r)
        nc.vector.tensor_copy(WprT[0:64, :], ps_w[0:64, 0:128])
        nc.vector.tensor_copy(WprT[64:128, :], ps_w[0:64, 128:256])
        WrpT = singles.tile([128, 128], f32r)
        nc.vector.tensor_copy(WrpT[0:64, :], ps_w[0:64, 128:256])
        nc.vector.tensor_copy(WrpT[64:128, :], ps_w[0:64, 0:128])
        ps_wh = ptr.tile([128, 512], f32, tag="ptr")
        nc.tensor.transpose(ps_wh[0:128, 0:1], whp_sb, one11)
        nc.tensor.transpose(ps_wh[0:128, 1:2], whr_sb, one11)
        whT = singles.tile([128, 2], f16)
        nc.vector.tensor_copy(whT, ps_wh[:, 0:2])
        whTr = singles.tile([128, 2], f16)
        nc.vector.tensor_copy(whTr[:, 0:1], ps_wh[:, 1:2])
        nc.vector.tensor_copy(whTr[:, 1:2], ps_wh[:, 0:1])

        st = {}

        # ---- phase A: loads, input transposes, At, fp8 copies --------------
        def phA(b):
            s = st[b] = {}
            s["RP"] = RP = sb.tile(name="rp", shape=[128, NT, 64], dtype=f32r, tag="rp")
            s["PP"] = PP = sb.tile(name="pp", shape=[128, NN, 64], dtype=f32r, tag="pp")
            nc.sync.dma_start(out=RP, in_=rv[b, :, :].rearrange("(i p) d -> p i d", p=128))
            nc.sync.dma_start(out=PP, in_=po[b, :, :].rearrange("(i p) d -> p i d", p=128))

            # fp8 + pooling copies (gpsimd)
            s["R8"] = R8 = sb.tile(name="r8", shape=[128, NT, 64], dtype=f8, tag="r8")
            nc.gpsimd.tensor_copy(out=R8, in_=RP)
            s["P8"] = P8 = sb.tile(name="p8", shape=[128, NN, 64], dtype=f8, tag="p8")
            nc.gpsimd.tensor_copy(out=P8, in_=PP)
            s["Pe"] = Pe = sb.tile(name="pe", shape=[128, NN, 65], dtype=f16, tag="pe")
            nc.gpsimd.tensor_copy(out=Pe[:, :, 0:64], in_=PP)
            nc.gpsimd.memset(Pe[:, :, 64:65], 1.0)
            s["Re"] = Re = sb.tile(name="re", shape=[128, NT, 65], dtype=f16, tag="re")
            nc.gpsimd.tensor_copy(out=Re[:, :, 0:64], in_=RP)
            nc.gpsimd.memset(Re[:, :, 64:65], 1.0)

            # transposes: R^T -> Hin_r[0:64], P^T -> Hin_p[0:64]
            s["Hin_r"] = Hin_r = sb.tile(name="hinr", shape=[128, 1024], dtype=f32r, tag="hinr")
            s["Hin_p"] = Hin_p = sb.tile(name="hinp", shape=[128, 1024], dtype=f32r, tag="hinp")
            for h in range(2):
                ps_r = ptr.tile([128, 512], f32r, tag="ptr")
                for i in range(4):
                    nc.tensor.transpose(ps_r[0:64, 128 * i:128 * (i + 1)],
                                        RP[:, 4 * h + i, :], ident)
                nc.vector.tensor_copy(Hin_r[0:64, 512 * h:512 * (h + 1)], ps_r[0:64, :])
            for h in range(2):
                ps_p = ptr.tile([128, 512], f32r, tag="ptr")
                for i in range(4):
                    nc.tensor.transpose(ps_p[0:64, 128 * i:128 * (i + 1)],
                                        PP[:, 4 * h + i, :], ident)
                nc.vector.tensor_copy(Hin_p[0:64, 512 * h:512 * (h + 1)], ps_p[0:64, :])

            # A^T = Wl^T @ R^T  -> At f32r (64, 1024)
            s["At"] = At = sb.tile(name="at", shape=[64, 1024], dtype=f32r, tag="at")
            for h in range(2):
                ps_a = ptr.tile([128, 512], f32, tag="ptr")
                nc.tensor.matmul(ps_a[0:64, :], wl_sb,
                                 Hin_r[0:64, 512 * h:512 * (h + 1)], start=True, stop=True)
                nc.vector.tensor_copy(At[:, 512 * h:512 * (h + 1)], ps_a[0:64, :])

        # ---- L tiles + trailing fp8 transposes -----------------------------
        def phL(b):
            s = st[b]
            At, Hin_p = s["At"], s["Hin_p"]
            s["Lf"] = Lf = sb.tile(name="lf", shape=[128, NT, 1024], dtype=f8, tag="lf")
            s["LT"] = LT = sb.tile(name="lt", shape=[128, NN, 1024], dtype=f8, tag="lt")

            def l_transposes(i):
                # fp8 transposes need output element step 2 (4-byte-aligned start)
                ps_lt = pt8.tile([128, NN, 256], f8, tag="pt8", name="ps_lt")
                for j in range(NN):
                    nc.tensor.transpose(ps_lt[:, j, 0:256:2], Lf[:, i, 128 * j:128 * (j + 1)], ident8)
                nc.vector.tensor_copy(LT[:, :, 128 * i:128 * (i + 1)], ps_lt[:, :, 0:256:2])

            for i in range(NT):
                for h in range(2):
                    ps_l = pl.tile([128, 512], f32, tag="pl")
                    nc.tensor.matmul(ps_l, At[:, 128 * i:128 * (i + 1)],
                                     Hin_p[0:64, 512 * h:512 * (h + 1)],
                                     start=True, stop=True)
                    nc.scalar.activation(Lf[:, i, 512 * h:512 * (h + 1)], ps_l, Tanh)
                if i > 0:
                    l_transposes(i - 1)
            l_transposes(NT - 1)

        # ---- X = R^T @ L ---------------------------------------------------
        def phX(b):
            s = st[b]
            R8, Lf, Hin_p = s["R8"], s["Lf"], s["Hin_p"]
            for c in range(2):
                ps_x = pxy.tile([64, 512], f32, tag="pxy")
                for q in range(4):
                    nc.tensor.matmul(ps_x, R8[:, 2 * q:2 * q + 2, :],
                                     Lf[:, 2 * q:2 * q + 2, 512 * c:512 * (c + 1)],
                                     start=(q == 0), stop=(q == 3), perf_mode=DR)
                nc.vector.tensor_copy(Hin_p[64:128, 512 * c:512 * (c + 1)], ps_x)

        # ---- Y = P^T @ L^T -------------------------------------------------
        def phY(b):
            s = st[b]
            P8, LT, Hin_r = s["P8"], s["LT"], s["Hin_r"]
            for c in range(2):
                ps_y = pxy.tile([64, 512], f32, tag="pxy")
                for q in range(4):
                    nc.tensor.matmul(ps_y, P8[:, 2 * q:2 * q + 2, :],
                                     LT[:, 2 * q:2 * q + 2, 512 * c:512 * (c + 1)],
                                     start=(q == 0), stop=(q == 3), perf_mode=DR)
                nc.vector.tensor_copy(Hin_r[64:128, 512 * c:512 * (c + 1)], ps_y)

        # ---- Hp / Hr -------------------------------------------------------
        def phH(b):
            s = st[b]
            Hin_p, Hin_r = s["Hin_p"], s["Hin_r"]
            s["Hp16"] = Hp16 = sb.tile(name="hp16", shape=[128, 1024], dtype=f16, tag="hp16")
            s["Hr16"] = Hr16 = sb.tile(name="hr16", shape=[128, 1024], dtype=f16, tag="hr16")
            for h in range(2):
                ps_h = pl.tile([128, 512], f32, tag="pl")
                nc.tensor.matmul(ps_h, WprT, Hin_p[:, 512 * h:512 * (h + 1)],
                                 start=True, stop=True)
                nc.scalar.activation(Hp16[:, 512 * h:512 * (h + 1)], ps_h, Tanh)
            for h in range(2):
                ps_h = pl.tile([128, 512], f32, tag="pl")
                nc.tensor.matmul(ps_h, WrpT, Hin_r[:, 512 * h:512 * (h + 1)],
                                 start=True, stop=True)
                nc.scalar.activation(Hr16[:, 512 * h:512 * (h + 1)], ps_h, Tanh)

        # ---- attention + pooling + output ----------------------------------
        def phC(b):
            s = st.pop(b)
            Hp16, Hr16 = s["Hp16"], s["Hr16"]
            Pe, Re = s["Pe"], s["Re"]
            eep = sb.tile([1, 1024], f16, tag="eep")
            eer = sb.tile([1, 1024], f16, tag="eer")
            for h in range(2):
                ps_lg = pl.tile([2, 512], f32, tag="pl")
                nc.tensor.matmul(ps_lg, whT, Hp16[:, 512 * h:512 * (h + 1)],
                                 start=True, stop=True)
                nc.scalar.activation(eep[0:1, 512 * h:512 * (h + 1)], ps_lg[0:1, :], Exp)
            for h in range(2):
                ps_lg = pl.tile([2, 512], f32, tag="pl")
                nc.tensor.matmul(ps_lg, whTr, Hr16[:, 512 * h:512 * (h + 1)],
                                 start=True, stop=True)
                nc.scalar.activation(eer[0:1, 512 * h:512 * (h + 1)], ps_lg[0:1, :], Exp)

            # transpose attention rows into per-tile columns (4-byte-aligned slots)
            ps_eet = pt8.tile([128, NN, 4], f16, tag="pt8")
            for j in range(NN):
                nc.tensor.transpose(ps_eet[:, j, 0:1], eep[:, 128 * j:128 * (j + 1)], one11h)
                nc.tensor.transpose(ps_eet[:, j, 2:3], eer[:, 128 * j:128 * (j + 1)], one11h)
            eeT = sb.tile([128, NN, 4], f16, tag="eet")
            nc.vector.tensor_copy(eeT, ps_eet)

            ps_cp = pxy.tile([64, 512], f32, tag="pxy")
            for j in range(NN):
                nc.tensor.matmul(ps_cp[0:1, 0:65], eeT[:, j, 0:1], Pe[:, j, :],
                                 start=(j == 0), stop=(j == NN - 1))
            ps_cr = pxy.tile([64, 512], f32, tag="pxy")
            for j in range(NT):
                nc.tensor.matmul(ps_cr[0:1, 0:65], eeT[:, j, 2:3], Re[:, j, :],
                                 start=(j == 0), stop=(j == NT - 1))

            rinv = sb.tile([1, 2], f32, tag="rinv")
            nc.vector.reciprocal(rinv[0:1, 0:1], ps_cp[0:1, 64:65])
            nc.vector.reciprocal(rinv[0:1, 1:2], ps_cr[0:1, 64:65])
            ob = sb.tile([1, 128], f32, tag="ob")
            nc.vector.tensor_scalar_mul(ob[0:1, 0:64], ps_cp[0:1, 0:64], rinv[0:1, 0:1])
            nc.vector.tensor_scalar_mul(ob[0:1, 64:128], ps_cr[0:1, 0:64], rinv[0:1, 1:2])
            nc.sync.dma_start(out=out[b:b + 1, :], in_=ob)

        # ---- software pipeline ---------------------------------------------
        phA(0)
        if BL > 1:
            phA(1)
        for k in range(BL):
            phL(k)
            phX(k)
            if k + 2 < BL:
                phA(k + 2)
            phY(k)
            phH(k)
            if k >= 1:
                phC(k - 1)
        phC(BL - 1)

    nc.compile()
    return nc


def get_nc():
    if "nc" not in _CACHE:
        _CACHE["nc"] = _build()
    return _CACHE["nc"]


def make_in_maps(inputs):
    R = np.ascontiguousarray(inputs["review_seq"], dtype=np.float32)
    P = np.ascontiguousarray(inputs["post_seq"], dtype=np.float32)
    w = {
        "Wl": np.ascontiguousarray(inputs["Wl"], dtype=np.float32),
        "Wr": np.ascontiguousarray(inputs["Wr"], dtype=np.float32),
        "Wp": np.ascontiguousarray(inputs["Wp"], dtype=np.float32),
        "whr": np.ascontiguousarray(inputs["whr"], dtype=np.float32),
        "whp": np.ascontiguousarray(inputs["whp"], dtype=np.float32),
    }
    in_maps = []
    for c in range(NCORES):
        m = {
            "review_seq": np.ascontiguousarray(R[c * BL:(c + 1) * BL]),
            "post_seq": np.ascontiguousarray(P[c * BL:(c + 1) * BL]),
        }
        m.update(w)
        in_maps.append(m)
    return in_maps


def run(inputs, trace=False):
    from concourse.bass_utils import run_bass_kernel_spmd

    nc = get_nc()
    res = run_bass_kernel_spmd(nc, make_in_maps(inputs),
                               core_ids=list(range(NCORES)), trace=trace)
    outp = np.concatenate([r["out"] for r in res.results], axis=0)
    return outp.astype(np.float32), res


def kernel(**inputs) -> np.ndarray:
    outp, _ = run(inputs, trace=False)
    return outp


# revision 20
# speedup vs baseline: 1.1264x; 1.1264x over previous
"""CoAttLayer Trainium2 kernel (v3: xbar transposes + fp8 DoubleRow X/Y).

Data-parallel over batch: 64 batches -> 8 NeuronCores x 8 batches.
Per batch (T = N = 1024, d = 64, k = 128):
    A  = R @ Wl                    [A^T = Wl^T @ R^T, f16]
    L  = tanh(A @ P^T)             (t, n)  f16 matmul -> tanh -> Lf16 (+fp8 copy)
    LT = L^T                       via DMA xbar transpose (fp16) (+fp8 copy)
    X  = R^T @ L                   fp8 DoubleRow over t-tile pairs
    Y  = P^T @ L^T                 fp8 DoubleRow over n-tile pairs
    Hp = tanh([Wp Wr] @ [P^T; X])  f16
    Hr = tanh([Wr Wp] @ [R^T; Y])  f16
    attention: row-matmul logits -> exp (accum_out gives the denominators)
    out = [P^T @ Ap ; R^T @ Ar]

Input R^T/P^T also via DMA xbar (single transpose per tensor using the
(128, tile, col) layout identity).  The PE runs only real matmuls; all big
transposes ride the DMA xbar; tanh(L) runs exactly once per element.
"""

import numpy as np
from contextlib import ExitStack

B, T, N, D, K = 64, 1024, 1024, 64, 128
NCORES = 8
BL = B // NCORES  # batches per core
NT = T // 128     # 8 t-tiles
NN = N // 128     # 8 n-tiles

_CACHE = {}


def _build():
    import concourse.tile as tile
    from concourse import bacc, mybir
    from concourse.masks import make_identity

    f32 = mybir.dt.float32
    f32r = mybir.dt.float32r
    f16 = mybir.dt.float16
    f8 = mybir.dt.float8e4
    Tanh = mybir.ActivationFunctionType.Tanh
    Exp = mybir.ActivationFunctionType.Exp
    DR = mybir.MatmulPerfMode.DoubleRow

    nc = bacc.Bacc(trn_type="TRN2")

    rv = nc.dram_tensor("review_seq", (BL, T, D), f32r, kind="ExternalInput")
    po = nc.dram_tensor("post_seq", (BL, N, D), f32r, kind="ExternalInput")
    wl = nc.dram_tensor("Wl", (D, D), f32r, kind="ExternalInput")
    wr = nc.dram_tensor("Wr", (K, D), f32r, kind="ExternalInput")
    wp = nc.dram_tensor("Wp", (K, D), f32r, kind="ExternalInput")
    whr = nc.dram_tensor("whr", (1, K), f32, kind="ExternalInput")
    whp = nc.dram_tensor("whp", (1, K), f32, kind="ExternalInput")
    out = nc.dram_tensor("out", (BL, 2 * D), f32, kind="ExternalOutput")
    import os
    DBG = bool(int(os.environ.get("KBDBG", "0")))
    if DBG:
        dbg = {
            "d_lf": nc.dram_tensor("d_lf", (128, NT, 1024), f16, kind="ExternalOutput"),
            "d_lt": nc.dram_tensor("d_lt", (128, NN, 1024), f8, kind="ExternalOutput"),
            "d_hinp": nc.dram_tensor("d_hinp", (128, 1024), f16, kind="ExternalOutput"),
            "d_hinr": nc.dram_tensor("d_hinr", (128, 1024), f16, kind="ExternalOutput"),
            "d_hp": nc.dram_tensor("d_hp", (128, 1024), f16, kind="ExternalOutput"),
            "d_hr": nc.dram_tensor("d_hr", (128, 1024), f16, kind="ExternalOutput"),
            "d_eep": nc.dram_tensor("d_eep", (1, 1024), f16, kind="ExternalOutput"),
            "d_eer": nc.dram_tensor("d_eer", (1, 1024), f16, kind="ExternalOutput"),
            "d_at": nc.dram_tensor("d_at", (64, 1024), f16, kind="ExternalOutput"),
            "d_esum": nc.dram_tensor("d_esum", (1, 4), f32, kind="ExternalOutput"),
        }

    with tile.TileContext(nc) as tc, ExitStack() as ctx:
        singles = ctx.enter_context(tc.tile_pool(name="singles", bufs=1))
        sb = ctx.enter_context(tc.tile_pool(name="sb", bufs=2))
        pl = ctx.enter_context(tc.tile_pool(name="pl", bufs=2, space="PSUM"))
        pxy = ctx.enter_context(tc.tile_pool(name="pxy", bufs=2, space="PSUM"))
        pee = ctx.enter_context(tc.tile_pool(name="pee", bufs=2, space="PSUM"))

        # ---- per-core constants -------------------------------------------
        ident32 = singles.tile([128, 128], f32)
        make_identity(nc, ident32)
        ident = singles.tile([128, 128], f32r)
        nc.vector.tensor_copy(ident, ident32)
        ident16 = singles.tile([128, 128], f16)
        nc.vector.tensor_copy(ident16, ident32)
        one11 = singles.tile([1, 1], f32)
        nc.vector.memset(one11, 1.0)
        one11h = singles.tile([1, 1], f16)
        nc.vector.memset(one11h, 1.0)

        wl_sb = singles.tile([64, 64], f32r)
        nc.sync.dma_start(out=wl_sb, in_=wl[:, :])
        wl16 = singles.tile([64, 64], f16)
        nc.vector.tensor_copy(wl16, wl_sb)
        wr_sb = singles.tile([128, 64], f32r)
        nc.sync.dma_start(out=wr_sb, in_=wr[:, :])
        wp_sb = singles.tile([128, 64], f32r)
        nc.sync.dma_start(out=wp_sb, in_=wp[:, :])
        whp_sb = singles.tile([1, 128], f32)
        nc.sync.dma_start(out=whp_sb, in_=whp[:, :])
        whr_sb = singles.tile([1, 128], f32)
        nc.sync.dma_start(out=whr_sb, in_=whr[:, :])

        # [Wp^T; Wr^T] / [Wr^T; Wp^T] f16 stacks; whT f16 columns
        ps_w = pl.tile([128, 512], f32r, tag="pl")
        nc.tensor.transpose(ps_w[0:64, 0:128], wp_sb, ident)
        nc.tensor.transpose(ps_w[0:64, 128:256], wr_sb, ident)
        WprT = singles.tile([128, 128], f16)
        nc.vector.tensor_copy(WprT[0:64, :], ps_w[0:64, 0:128])
        nc.vector.tensor_copy(WprT[64:128, :], ps_w[0:64, 128:256])
        WrpT = singles.tile([128, 128], f16)
        nc.vector.tensor_copy(WrpT[0:64, :], ps_w[0:64, 128:256])
        nc.vector.tensor_copy(WrpT[64:128, :], ps_w[0:64, 0:128])
        ps_wh = pl.tile([128, 512], f32, tag="pl")
        nc.tensor.transpose(ps_wh[0:128, 0:1], whp_sb, one11)
        nc.tensor.transpose(ps_wh[0:128, 1:2], whr_sb, one11)
        whT = singles.tile([128, 2], f16)
        nc.vector.tensor_copy(whT, ps_wh[:, 0:2])
        whTr = singles.tile([128, 2], f16)
        nc.vector.tensor_copy(whTr[:, 0:1], ps_wh[:, 1:2])
        nc.vector.tensor_copy(whTr[:, 1:2], ps_wh[:, 0:1])

        st = {}

        # ---- phase A: loads, casts, xbar input transposes, At --------------
        def phA(b):
            s = st[b] = {}
            s["RP"] = RP = sb.tile(name="rp", shape=[128, NT, 64], dtype=f32r, tag="rp")
            s["PP"] = PP = sb.tile(name="pp", shape=[128, NN, 64], dtype=f32r, tag="pp")
            nc.sync.dma_start(out=RP, in_=rv[b, :, :].rearrange("(i p) d -> p i d", p=128))
            nc.sync.dma_start(out=PP, in_=po[b, :, :].rearrange("(i p) d -> p i d", p=128))

            # f16 copies: full-tile single-instruction writes (the xbar read
            # races any partial/strided writer of its source tile)
            s["R16"] = R16 = sb.tile(name="r16", shape=[128, NT, 64], dtype=f16, tag="r16", bufs=3)
            nc.gpsimd.tensor_copy(out=R16, in_=RP)
            s["P16"] = P16 = sb.tile(name="p16", shape=[128, NN, 64], dtype=f16, tag="p16", bufs=3)
            nc.gpsimd.tensor_copy(out=P16, in_=PP)
            s["R8"] = R8 = sb.tile(name="r8", shape=[128, NT, 64], dtype=f8, tag="r8", bufs=3)
            nc.vector.tensor_copy(R8, RP)
            s["P8"] = P8 = sb.tile(name="p8", shape=[128, NN, 64], dtype=f8, tag="p8", bufs=3)
            nc.vector.tensor_copy(P8, PP)

            # input transposes on the PE (f32r), evacuated as f16
            s["Hin_r"] = Hin_r = sb.tile(name="hinr", shape=[128, 1024], dtype=f16, tag="hinr", bufs=3)
            s["Hin_p"] = Hin_p = sb.tile(name="hinp", shape=[128, 1024], dtype=f16, tag="hinp", bufs=3)
            for h in range(2):
                ps_r = pl.tile([128, 512], f32r, tag="pl", name="ps_r")
                for i in range(4):
                    nc.tensor.transpose(ps_r[0:64, 128 * i:128 * (i + 1)],
                                        RP[:, 4 * h + i, :], ident)
                nc.vector.tensor_copy(Hin_r[0:64, 512 * h:512 * (h + 1)], ps_r[0:64, :])
            for h in range(2):
                ps_p = pl.tile([128, 512], f32r, tag="pl", name="ps_p")
                for i in range(4):
                    nc.tensor.transpose(ps_p[0:64, 128 * i:128 * (i + 1)],
                                        PP[:, 4 * h + i, :], ident)
                nc.vector.tensor_copy(Hin_p[0:64, 512 * h:512 * (h + 1)], ps_p[0:64, :])

            # A^T = Wl^T @ R^T (f16)
            s["At"] = At = sb.tile(name="at", shape=[64, 1024], dtype=f16, tag="at", bufs=3)
            for h in range(2):
                ps_a = pl.tile([128, 512], f32, tag="pl")
                nc.tensor.matmul(ps_a[0:64, :], wl16,
                                 Hin_r[0:64, 512 * h:512 * (h + 1)], start=True, stop=True)
                nc.vector.tensor_copy(At[:, 512 * h:512 * (h + 1)], ps_a[0:64, :])

        # ---- L tiles: matmul+tanh, fp8 copy, xbar L^T, fp8 copy ------------
        def phL(b):
            s = st[b]
            At, Hin_p = s["At"], s["Hin_p"]
            s["Lf16"] = Lf16 = sb.tile(name="lf16", shape=[128, NT, 1024], dtype=f16, tag="lf16")
            s["Lf8"] = Lf8 = sb.tile(name="lf8", shape=[128, NT, 1024], dtype=f8, tag="lf8")

            s["LT8"] = LT8 = sb.tile(name="lt8", shape=[128, NN, 1024], dtype=f8, tag="lt8")
            for i in range(NT):
                ps_l = pl.tile([128, 1024], f32, tag="pl")
                for h in range(2):
                    nc.tensor.matmul(ps_l[:, 512 * h:512 * (h + 1)],
                                     At[:, 128 * i:128 * (i + 1)],
                                     Hin_p[0:64, 512 * h:512 * (h + 1)],
                                     start=True, stop=True)
                nc.scalar.activation(Lf16[:, i, :], ps_l, Tanh)
                nc.vector.tensor_copy(Lf8[:, i, :], Lf16[:, i, :])
                ps_lt = pee.tile([128, NN, 128], f16, tag="plt", name="ps_lt")
                for j in range(NN):
                    nc.tensor.transpose(ps_lt[:, j, :], Lf16[:, i, 128 * j:128 * (j + 1)],
                                        ident16)
                nc.vector.tensor_copy(LT8[:, :, 128 * i:128 * (i + 1)], ps_lt)

        # ---- X = R^T @ L (fp8 DoubleRow) -----------------------------------
        def phX(b):
            s = st[b]
            R8, Lf8, Hin_p = s["R8"], s["Lf8"], s["Hin_p"]
            for c in range(2):
                ps_x = pxy.tile([64, 512], f32, tag="pxy")
                for q in range(4):
                    nc.tensor.matmul(ps_x, R8[:, 2 * q:2 * q + 2, :],
                                     Lf8[:, 2 * q:2 * q + 2, 512 * c:512 * (c + 1)],
                                     start=(q == 0), stop=(q == 3), perf_mode=DR)
                nc.vector.tensor_copy(Hin_p[64:128, 512 * c:512 * (c + 1)], ps_x)

        # ---- Y = P^T @ L^T (fp8 DoubleRow) ---------------------------------
        def phY(b):
            s = st[b]
            P8, LT8, Hin_r = s["P8"], s["LT8"], s["Hin_r"]
            for c in range(2):
                ps_y = pxy.tile([64, 512], f32, tag="pxy")
                for q in range(4):
                    nc.tensor.matmul(ps_y, P8[:, 2 * q:2 * q + 2, :],
                                     LT8[:, 2 * q:2 * q + 2, 512 * c:512 * (c + 1)],
                                     start=(q == 0), stop=(q == 3), perf_mode=DR)
                nc.vector.tensor_copy(Hin_r[64:128, 512 * c:512 * (c + 1)], ps_y)

        # ---- Hp / Hr -------------------------------------------------------
        def phH(b):
            s = st[b]
            Hin_p, Hin_r = s["Hin_p"], s["Hin_r"]
            s["Hp16"] = Hp16 = sb.tile(name="hp16", shape=[128, 1024], dtype=f16, tag="hp16")
            s["Hr16"] = Hr16 = sb.tile(name="hr16", shape=[128, 1024], dtype=f16, tag="hr16")
            for h in range(2):
                ps_h = pl.tile([128, 512], f32, tag="pl")
                nc.tensor.matmul(ps_h, WprT, Hin_p[:, 512 * h:512 * (h + 1)],
                                 start=True, stop=True)
                nc.scalar.activation(Hp16[:, 512 * h:512 * (h + 1)], ps_h, Tanh)
            for h in range(2):
                ps_h = pl.tile([128, 512], f32, tag="pl")
                nc.tensor.matmul(ps_h, WrpT, Hin_r[:, 512 * h:512 * (h + 1)],
                                 start=True, stop=True)
                nc.scalar.activation(Hr16[:, 512 * h:512 * (h + 1)], ps_h, Tanh)

        # ---- attention + pooling + output ----------------------------------
        def phC(b):
            s = st.pop(b)
            Hp16, Hr16 = s["Hp16"], s["Hr16"]
            R16, P16 = s["R16"], s["P16"]
            eep = sb.tile([1, 1024], f16, tag="eep")
            eer = sb.tile([1, 1024], f16, tag="eer")
            esum = sb.tile([1, 4], f32, tag="esum")
            for h in range(2):
                ps_lg = pl.tile([2, 512], f32, tag="pl")
                nc.tensor.matmul(ps_lg, whT, Hp16[:, 512 * h:512 * (h + 1)],
                                 start=True, stop=True)
                nc.scalar.activation(eep[0:1, 512 * h:512 * (h + 1)], ps_lg[0:1, :], Exp,
                                     accum_out=esum[0:1, h:h + 1])
            for h in range(2):
                ps_lg = pl.tile([2, 512], f32, tag="pl")
                nc.tensor.matmul(ps_lg, whTr, Hr16[:, 512 * h:512 * (h + 1)],
                                 start=True, stop=True)
                nc.scalar.activation(eer[0:1, 512 * h:512 * (h + 1)], ps_lg[0:1, :], Exp,
                                     accum_out=esum[0:1, 2 + h:3 + h])

            # attention rows -> per-tile columns (PE transposes, 4B-aligned)
            ps_eet = pee.tile([128, NN, 4], f16, tag="plt")
            for j in range(NN):
                nc.tensor.transpose(ps_eet[:, j, 0:1], eep[:, 128 * j:128 * (j + 1)], one11h)
                nc.tensor.transpose(ps_eet[:, j, 2:3], eer[:, 128 * j:128 * (j + 1)], one11h)
            eeT = sb.tile([128, NN, 4], f16, tag="eet")
            nc.vector.tensor_copy(eeT, ps_eet)

            ps_cp = pxy.tile([64, 512], f32, tag="pxy")
            for j in range(NN):
                nc.tensor.matmul(ps_cp[0:1, 0:64], eeT[:, j, 0:1], P16[:, j, :],
                                 start=(j == 0), stop=(j == NN - 1))
            ps_cr = pxy.tile([64, 512], f32, tag="pxy")
            for j in range(NT):
                nc.tensor.matmul(ps_cr[0:1, 0:64], eeT[:, j, 2:3], R16[:, j, :],
                                 start=(j == 0), stop=(j == NT - 1))

            if DBG and b == 0:
                nc.sync.dma_start(out=dbg["d_lf"][:, :, :], in_=s["Lf16"])
                nc.sync.dma_start(out=dbg["d_lt"][:, :, :], in_=s["LT8"])
                nc.sync.dma_start(out=dbg["d_hinp"][:, :], in_=s["Hin_p"])
                nc.sync.dma_start(out=dbg["d_hinr"][:, :], in_=s["Hin_r"])
                nc.sync.dma_start(out=dbg["d_hp"][:, :], in_=Hp16)
                nc.sync.dma_start(out=dbg["d_hr"][:, :], in_=Hr16)
                nc.sync.dma_start(out=dbg["d_eep"][:, :], in_=eep)
                nc.sync.dma_start(out=dbg["d_eer"][:, :], in_=eer)
                nc.sync.dma_start(out=dbg["d_at"][:, :], in_=s["At"])
                nc.sync.dma_start(out=dbg["d_esum"][:, :], in_=esum)
            den = sb.tile([1, 2], f32, tag="den")
            nc.vector.tensor_tensor(out=den[0:1, 0:1], in0=esum[0:1, 0:1],
                                    in1=esum[0:1, 1:2], op=mybir.AluOpType.add)
            nc.vector.tensor_tensor(out=den[0:1, 1:2], in0=esum[0:1, 2:3],
                                    in1=esum[0:1, 3:4], op=mybir.AluOpType.add)
            rinv = sb.tile([1, 2], f32, tag="rinv")
            nc.vector.reciprocal(rinv, den)
            ob = sb.tile([1, 128], f32, tag="ob")
            nc.vector.tensor_scalar_mul(ob[0:1, 0:64], ps_cp[0:1, 0:64], rinv[0:1, 0:1])
            nc.vector.tensor_scalar_mul(ob[0:1, 64:128], ps_cr[0:1, 0:64], rinv[0:1, 1:2])
            nc.sync.dma_start(out=out[b:b + 1, :], in_=ob)

        # ---- software pipeline ---------------------------------------------
        phA(0)
        if BL > 1:
            phA(1)
        for k in range(BL):
            phL(k)
            phX(k)
            if k >= 1:
                phC(k - 1)
            if k + 2 < BL:
                phA(k + 2)
            phY(k)
            phH(k)
        phC(BL - 1)

    nc.compile()
    return nc


def get_nc():
    if "nc" not in _CACHE:
        _CACHE["nc"] = _build()
    return _CACHE["nc"]


def make_in_maps(inputs):
    R = np.ascontiguousarray(inputs["review_seq"], dtype=np.float32)
    P = np.ascontiguousarray(inputs["post_seq"], dtype=np.float32)
    w = {
        "Wl": np.ascontiguousarray(inputs["Wl"], dtype=np.float32),
        "Wr": np.ascontiguousarray(inputs["Wr"], dtype=np.float32),
        "Wp": np.ascontiguousarray(inputs["Wp"], dtype=np.float32),
        "whr": np.ascontiguousarray(inputs["whr"], dtype=np.float32),
        "whp": np.ascontiguousarray(inputs["whp"], dtype=np.float32),
    }
    in_maps = []
    for c in range(NCORES):
        m = {
            "review_seq": np.ascontiguousarray(R[c * BL:(c + 1) * BL]),
            "post_seq": np.ascontiguousarray(P[c * BL:(c + 1) * BL]),
        }
        m.update(w)
        in_maps.append(m)
    return in_maps


def run(inputs, trace=False):
    from concourse.bass_utils import run_bass_kernel_spmd

    nc = get_nc()
    res = run_bass_kernel_spmd(nc, make_in_maps(inputs),
                               core_ids=list(range(NCORES)), trace=trace)
    outp = np.concatenate([r["out"] for r in res.results], axis=0)
    return outp.astype(np.float32), res


def kernel(**inputs) -> np.ndarray:
    outp, _ = run(inputs, trace=False)
    return outp


# revision 21
# speedup vs baseline: 1.4077x; 1.2497x over previous
"""CoAttLayer Trainium2 kernel.

Data-parallel over batch: 64 batches -> 8 NeuronCores x 8 batches.
Per batch (T = N = 1024, d = 64, k = 128):
    L  = tanh(R @ Wl @ P^T)                      (T, N)
    Hp = tanh(Wp @ P^T + (Wr @ R^T) @ L)         (k, N)
    Hr = tanh(Wr @ R^T + (Wp @ P^T) @ L^T)       (k, T)
    Ap = softmax(whp @ Hp), Ar = softmax(whr @ Hr)
    out = [P^T @ Ap ; R^T @ Ar]                  (2d,)

Layout strategy: all d-contractions run on partitions 0-63 (R^T, P^T, A^T and
the transposed small weights all live there).  L is produced tile-wise in PSUM
(t on partitions), tanh'd by ScalarE straight into fp16 SBUF, and L^T is
produced by the DMA xbar transpose (fp16) so neither the PE nor the DVE pays
for the big transpose.  Big matmuls run in float32r (full PE rate at free-dim
512); the L-sized operands run in fp16.
"""

import numpy as np
from contextlib import ExitStack

B, T, N, D, K = 64, 1024, 1024, 64, 128
NCORES = 8
BL = B // NCORES  # batches per core

_CACHE = {}


def _build():
    import concourse.tile as tile
    from concourse import bacc, mybir
    from concourse.masks import make_identity

    f32 = mybir.dt.float32
    f32r = mybir.dt.float32r
    f16 = mybir.dt.float16
    Tanh = mybir.ActivationFunctionType.Tanh
    Exp = mybir.ActivationFunctionType.Exp

    nc = bacc.Bacc(trn_type="TRN2")

    rv = nc.dram_tensor("review_seq", (BL, T, D), f32r, kind="ExternalInput")
    po = nc.dram_tensor("post_seq", (BL, N, D), f32r, kind="ExternalInput")
    wl = nc.dram_tensor("Wl", (D, D), f32r, kind="ExternalInput")
    wr = nc.dram_tensor("Wr", (K, D), f32r, kind="ExternalInput")
    wp = nc.dram_tensor("Wp", (K, D), f32r, kind="ExternalInput")
    whr = nc.dram_tensor("whr", (1, K), f32, kind="ExternalInput")
    whp = nc.dram_tensor("whp", (1, K), f32, kind="ExternalInput")
    out = nc.dram_tensor("out", (BL, 2 * D), f32, kind="ExternalOutput")
    import os
    DBG = bool(int(os.environ.get("KBDBG", "0")))
    if DBG:
        dbg_lf = nc.dram_tensor("dbg_lf", (BL, 128, 8, 1024), f16, kind="ExternalOutput")
        dbg_lt = nc.dram_tensor("dbg_lt", (BL, 128, 8, 1024), f16, kind="ExternalOutput")
        dbg_hp = nc.dram_tensor("dbg_hp", (BL, 128, 1024), f16, kind="ExternalOutput")
        dbg_hr = nc.dram_tensor("dbg_hr", (BL, 128, 1024), f16, kind="ExternalOutput")
        dbg_ee = nc.dram_tensor("dbg_ee", (BL, 128, 16), f16, kind="ExternalOutput")

    NT = T // 128  # 8 t-tiles
    NN = N // 128  # 8 n-tiles

    with tile.TileContext(nc) as tc, ExitStack() as ctx:
        singles = ctx.enter_context(tc.tile_pool(name="singles", bufs=1))
        sb = ctx.enter_context(tc.tile_pool(name="sb", bufs=2))
        pa = ctx.enter_context(tc.tile_pool(name="pa", bufs=2, space="PSUM"))
        pb = ctx.enter_context(tc.tile_pool(name="pb", bufs=2, space="PSUM"))

        # ---- per-core constants -------------------------------------------
        ident32 = singles.tile([128, 128], f32)
        make_identity(nc, ident32)
        ident = singles.tile([128, 128], f32r)
        nc.vector.tensor_copy(ident, ident32)
        one11 = singles.tile([1, 1], f32)
        nc.vector.memset(one11, 1.0)
        ident16 = singles.tile([128, 128], f16)
        nc.vector.tensor_copy(ident16, ident32)

        wl_sb = singles.tile([64, 64], f32r)
        nc.sync.dma_start(out=wl_sb, in_=wl[:, :])
        wl16 = singles.tile([64, 64], f16)
        nc.vector.tensor_copy(wl16, wl_sb)
        wr_sb = singles.tile([128, 64], f32r)
        nc.sync.dma_start(out=wr_sb, in_=wr[:, :])
        wp_sb = singles.tile([128, 64], f32r)
        nc.sync.dma_start(out=wp_sb, in_=wp[:, :])
        whp_sb = singles.tile([1, 128], f32)
        nc.sync.dma_start(out=whp_sb, in_=whp[:, :])
        whr_sb = singles.tile([1, 128], f32)
        nc.sync.dma_start(out=whr_sb, in_=whr[:, :])

        # Wr^T, Wp^T on partitions 0-63; whp^T/whr^T as fp16 columns.
        ps_w = pa.tile([128, 1024], f32r, tag="pa")
        nc.tensor.transpose(ps_w[0:64, 0:128], wr_sb, ident)
        nc.tensor.transpose(ps_w[0:64, 128:256], wp_sb, ident)
        wrT = singles.tile([64, 128], f16)
        nc.vector.tensor_copy(wrT, ps_w[0:64, 0:128])
        wpT = singles.tile([64, 128], f16)
        nc.vector.tensor_copy(wpT, ps_w[0:64, 128:256])
        ps_wh = pa.tile([128, 2], f32, tag="pa")
        nc.tensor.transpose(ps_wh[0:128, 0:1], whp_sb, one11)
        nc.tensor.transpose(ps_wh[0:128, 1:2], whr_sb, one11)
        whT = singles.tile([128, 2], f16)
        nc.vector.tensor_copy(whT, ps_wh)

        # ---- per-batch pipeline, software-pipelined emission ---------------
        # Sub-phases interleaved across three consecutive batches so each
        # engine's in-order stream has its dependencies ready just-in-time:
        #   A1: loads + input transposes + Rt/Pt evacuation
        #   A2: A^T, G_r, G_p matmuls + their fp16 casts
        #   A3: G transposes + evacuation, Pe/Re prep (gpsimd)
        #   B1: L tiles (matmul+tanh) with L^T transposes trailing one tile
        #   B2: Hp accumulation + tanh      B3: Hr accumulation + tanh
        #   C : logits, exp, pooling, output
        # Emission per iteration k: A1(k+2) B1(k+1) A2(k+2) B2(k+1) A3(k+2)
        # B3(k+1) C(k).
        st = {}

        def phaseA1(b):
            s = st[b] = {}
            s["RP"] = RP = sb.tile(name="rp", shape=[128, NT, 64], dtype=f32r, tag="rp", bufs=3)
            s["PP"] = PP = sb.tile(name="pp", shape=[128, NN, 64], dtype=f32r, tag="pp", bufs=3)
            nc.sync.dma_start(out=RP, in_=rv[b, :, :].rearrange("(i p) d -> p i d", p=128))
            nc.sync.dma_start(out=PP, in_=po[b, :, :].rearrange("(i p) d -> p i d", p=128))

            ps_rt = pa.tile([128, 1024], f32r, tag="pa")
            for i in range(NT):
                nc.tensor.transpose(ps_rt[0:64, 128 * i:128 * (i + 1)], RP[:, i, :], ident)
            s["Rt"] = Rt = sb.tile(name="rt", shape=[64, 1024], dtype=f16, tag="rt", bufs=3)
            nc.vector.tensor_copy(Rt, ps_rt[0:64, :])

            ps_pt = pa.tile([128, 1024], f32r, tag="pa")
            for i in range(NN):
                nc.tensor.transpose(ps_pt[0:64, 128 * i:128 * (i + 1)], PP[:, i, :], ident)
            s["Pt"] = Pt = sb.tile(name="pt", shape=[64, 1024], dtype=f16, tag="pt", bufs=3)
            nc.vector.tensor_copy(Pt, ps_pt[0:64, :])

        def phaseA2(b):
            s = st[b]
            Rt, Pt = s["Rt"], s["Pt"]
            ps_at = pa.tile([128, 1024], f32, tag="pa")
            nc.tensor.matmul(ps_at[0:64, 0:512], wl16, Rt[:, 0:512], start=True, stop=True)
            nc.tensor.matmul(ps_at[0:64, 512:1024], wl16, Rt[:, 512:1024], start=True, stop=True)
            ps_gr = pb.tile([128, 1024], f32, tag="pb")
            nc.tensor.matmul(ps_gr[:, 0:512], wrT, Rt[:, 0:512], start=True, stop=True)
            nc.tensor.matmul(ps_gr[:, 512:1024], wrT, Rt[:, 512:1024], start=True, stop=True)
            ps_gp = pb.tile([128, 1024], f32, tag="pb")
            nc.tensor.matmul(ps_gp[:, 0:512], wpT, Pt[:, 0:512], start=True, stop=True)
            nc.tensor.matmul(ps_gp[:, 512:1024], wpT, Pt[:, 512:1024], start=True, stop=True)
            s["AT"] = AT = sb.tile(name="at", shape=[64, 1024], dtype=f16, tag="at", bufs=3)
            nc.vector.tensor_copy(AT, ps_at[0:64, :])
            s["Gr16"] = Gr16 = sb.tile(name="gr16", shape=[128, 1024], dtype=f16, tag="gr16", bufs=3)
            nc.vector.tensor_copy(Gr16, ps_gr)
            s["Gp16"] = Gp16 = sb.tile(name="gp16", shape=[128, 1024], dtype=f16, tag="gp16", bufs=3)
            nc.vector.tensor_copy(Gp16, ps_gp)

        def phaseA3(b):
            s = st[b]
            RP, PP = s["RP"], s["PP"]
            Gr16, Gp16 = s.pop("Gr16"), s.pop("Gp16")
            # both transpose sets share one PSUM tile and one evacuation
            s["GT"] = GT = sb.tile(name="gt", shape=[128, NT + NN, 128], dtype=f16, tag="gt", bufs=3)
            ps_gt = pb.tile([128, NT + NN, 128], f16, tag="pb")
            for a in range(NT):
                nc.tensor.transpose(ps_gt[:, a, :], Gr16[:, 128 * a:128 * (a + 1)], ident16)
            for a in range(NN):
                nc.tensor.transpose(ps_gt[:, NT + a, :], Gp16[:, 128 * a:128 * (a + 1)], ident16)
            nc.vector.tensor_copy(GT, ps_gt)

            # pooling rhs with ones column (gpsimd: off the DVE)
            s["Pe"] = Pe = sb.tile(name="pe", shape=[128, NN, 65], dtype=f16, tag="pe", bufs=3)
            nc.gpsimd.tensor_copy(out=Pe[:, :, 0:64], in_=PP)
            nc.gpsimd.memset(Pe[:, :, 64:65], 1.0)
            s["Re"] = Re = sb.tile(name="re", shape=[128, NT, 65], dtype=f16, tag="re", bufs=3)
            nc.gpsimd.tensor_copy(out=Re[:, :, 0:64], in_=RP)
            nc.gpsimd.memset(Re[:, :, 64:65], 1.0)

        def phaseB1(b):
            s = st[b]
            Pt, AT = s["Pt"], s["AT"]
            # L tiles: L_i = tanh(A_i @ P^T) -> fp16 ; L^T via PE transposes
            s["Lf"] = Lf = sb.tile(name="lf", shape=[128, NT, 1024], dtype=f16, tag="lf")
            s["LT"] = LT = sb.tile(name="lt", shape=[128, NN, 1024], dtype=f16, tag="lt")

            def l_transposes(i):
                ps_lt = pa.tile([128, NN, 128], f16, tag="pa")
                for j in range(NN):
                    nc.tensor.transpose(ps_lt[:, j, :], Lf[:, i, 128 * j:128 * (j + 1)], ident16)
                nc.vector.tensor_copy(LT[:, :, 128 * i:128 * (i + 1)], ps_lt)

            for i in range(NT):
                ps_l = pa.tile([128, 1024], f32, tag="pa")
                lhs = AT[:, 128 * i:128 * (i + 1)]
                nc.tensor.matmul(ps_l[:, 0:512], lhs, Pt[:, 0:512], start=True, stop=True)
                nc.tensor.matmul(ps_l[:, 512:1024], lhs, Pt[:, 512:1024], start=True, stop=True)
                nc.scalar.activation(Lf[:, i, :], ps_l, Tanh)
                if i > 0:
                    l_transposes(i - 1)
            l_transposes(NT - 1)

        def phaseB2(b):
            s = st[b]
            Pt, GT, Lf = s["Pt"], s["GT"], s["Lf"]
            # Hp = tanh(G_p + sum_t G_r^T.T @ L)   (k, n)
            ps_hp = pb.tile([128, 1024], f32, tag="pb")
            nc.tensor.matmul(ps_hp[:, 0:512], wpT, Pt[:, 0:512], start=True, stop=False)
            nc.tensor.matmul(ps_hp[:, 512:1024], wpT, Pt[:, 512:1024], start=True, stop=False)
            for j in range(NT):
                nc.tensor.matmul(ps_hp[:, 0:512], GT[:, j, :], Lf[:, j, 0:512],
                                 start=False, stop=(j == NT - 1))
                nc.tensor.matmul(ps_hp[:, 512:1024], GT[:, j, :], Lf[:, j, 512:1024],
                                 start=False, stop=(j == NT - 1))
            s["Hp16"] = Hp16 = sb.tile(name="hp16", shape=[128, 1024], dtype=f16, tag="hp16")
            nc.scalar.activation(Hp16, ps_hp, Tanh)

        def phaseB3(b):
            s = st[b]
            Rt, GT, LT = s["Rt"], s["GT"], s["LT"]
            # Hr = tanh(G_r + sum_n G_p^T.T @ L^T)   (k, t)
            ps_hr = pb.tile([128, 1024], f32, tag="pb")
            nc.tensor.matmul(ps_hr[:, 0:512], wrT, Rt[:, 0:512], start=True, stop=False)
            nc.tensor.matmul(ps_hr[:, 512:1024], wrT, Rt[:, 512:1024], start=True, stop=False)
            for j in range(NN):
                nc.tensor.matmul(ps_hr[:, 0:512], GT[:, NT + j, :], LT[:, j, 0:512],
                                 start=False, stop=(j == NN - 1))
                nc.tensor.matmul(ps_hr[:, 512:1024], GT[:, NT + j, :], LT[:, j, 512:1024],
                                 start=False, stop=(j == NN - 1))
            s["Hr16"] = Hr16 = sb.tile(name="hr16", shape=[128, 1024], dtype=f16, tag="hr16")
            nc.scalar.activation(Hr16, ps_hr, Tanh)

        def phaseC(b):
            s = st.pop(b)
            Hp16, Hr16 = s["Hp16"], s["Hr16"]
            Pe, Re = s["Pe"], s["Re"]
            Lf, LT = s["Lf"], s["LT"]
            # logits^T: (n,1) and (t,1) per 128-chunk, then exp (no max-sub:
            # |logit| <= ||wh||_1 ~ 5, exp stays in fp16 range)
            ps_lg = pa.tile([128, 16], f32, tag="pa")
            for i in range(NN):
                nc.tensor.matmul(ps_lg[:, i:i + 1], Hp16[:, 128 * i:128 * (i + 1)],
                                 whT[:, 0:1], start=True, stop=True)
            for i in range(NT):
                nc.tensor.matmul(ps_lg[:, 8 + i:9 + i], Hr16[:, 128 * i:128 * (i + 1)],
                                 whT[:, 1:2], start=True, stop=True)
            ee = sb.tile([128, 16], f16, tag="ee")
            nc.scalar.activation(ee, ps_lg, Exp)

            ps_co = pa.tile([128, 1024], f32, tag="pa")
            for j in range(NN):
                nc.tensor.matmul(ps_co[0:1, 0:65], ee[:, j:j + 1], Pe[:, j, :],
                                 start=(j == 0), stop=(j == NN - 1))
            for j in range(NT):
                nc.tensor.matmul(ps_co[0:1, 512:577], ee[:, 8 + j:9 + j], Re[:, j, :],
                                 start=(j == 0), stop=(j == NT - 1))

            if DBG:
                nc.sync.dma_start(out=dbg_lf[b], in_=Lf)
                nc.sync.dma_start(out=dbg_lt[b], in_=LT)
                nc.sync.dma_start(out=dbg_hp[b], in_=Hp16)
                nc.sync.dma_start(out=dbg_hr[b], in_=Hr16)
                nc.sync.dma_start(out=dbg_ee[b], in_=ee)
            rinv = sb.tile([1, 2], f32, tag="rinv")
            nc.vector.reciprocal(rinv[0:1, 0:1], ps_co[0:1, 64:65])
            nc.vector.reciprocal(rinv[0:1, 1:2], ps_co[0:1, 576:577])
            ob = sb.tile([1, 128], f32, tag="ob")
            nc.vector.tensor_scalar_mul(ob[0:1, 0:64], ps_co[0:1, 0:64], rinv[0:1, 0:1])
            nc.vector.tensor_scalar_mul(ob[0:1, 64:128], ps_co[0:1, 512:576], rinv[0:1, 1:2])
            nc.sync.dma_start(out=out[b:b + 1, :], in_=ob)

        def fullA(b):
            phaseA1(b); phaseA2(b); phaseA3(b)

        fullA(0)
        if BL > 1:
            phaseA1(1)
            phaseB1(0)
            phaseA2(1)
            phaseB2(0)
            phaseA3(1)
            phaseB3(0)
        else:
            phaseB1(0); phaseB2(0); phaseB3(0)
        for k in range(BL):
            if k + 2 < BL:
                phaseA1(k + 2)
            if k + 1 < BL:
                phaseB1(k + 1)
            if k + 2 < BL:
                phaseA2(k + 2)
            if k + 1 < BL:
                phaseB2(k + 1)
            if k + 2 < BL:
                phaseA3(k + 2)
            if k + 1 < BL:
                phaseB3(k + 1)
            phaseC(k)

    nc.compile()
    return nc


def get_nc():
    if "nc" not in _CACHE:
        _CACHE["nc"] = _build()
    return _CACHE["nc"]


def make_in_maps(inputs):
    R = np.ascontiguousarray(inputs["review_seq"], dtype=np.float32)
    P = np.ascontiguousarray(inputs["post_seq"], dtype=np.float32)
    w = {
        "Wl": np.ascontiguousarray(inputs["Wl"], dtype=np.float32),
        "Wr": np.ascontiguousarray(inputs["Wr"], dtype=np.float32),
        "Wp": np.ascontiguousarray(inputs["Wp"], dtype=np.float32),
        "whr": np.ascontiguousarray(inputs["whr"], dtype=np.float32),
        "whp": np.ascontiguousarray(inputs["whp"], dtype=np.float32),
    }
    in_maps = []
    for c in range(NCORES):
        m = {
            "review_seq": np.ascontiguousarray(R[c * BL:(c + 1) * BL]),
            "post_seq": np.ascontiguousarray(P[c * BL:(c + 1) * BL]),
        }
        m.update(w)
        in_maps.append(m)
    return in_maps


def run(inputs, trace=False):
    from concourse.bass_utils import run_bass_kernel_spmd

    nc = get_nc()
    res = run_bass_kernel_spmd(nc, make_in_maps(inputs),
                               core_ids=list(range(NCORES)), trace=trace)
    outp = np.concatenate([r["out"] for r in res.results], axis=0)
    return outp.astype(np.float32), res


def kernel(**inputs) -> np.ndarray:
    outp, _ = run(inputs, trace=False)
    return outp

